# revision 1
# baseline (speedup 1.0000x reference)
"""Trainium2 Bass kernel for nn_BornFoward: 200-step leapfrog wave recurrence.

Math (validated against the jax reference in a numpy model):
  - coef = (dt*BGF/dx)^2 is 0.2025 in the interior square [25:167)^2 of the
    192x192 grid and ~4.4e-13 in the outer absorbing ring; rf is EXACTLY zero
    outside the central 96x96 window (pad region has X==1 -> 1-X^2==0).
  - Therefore the recurrence restricted to the 142x142 interior with zero
    Dirichlet boundary and constant coef reproduces the reference to ~1e-9.
  - p_new = 2*p1 - p0 + C*lap4(p1) + rf*d2(P0),  meas = p_new at 32 pixels.

Sharding: 16 independent recurrences (B=2 x NR=8) -> channel r per core,
both batches per core, batched along the matmul free (column) dimension.

Layout per core: state tiles [71 partitions, 2 chunks x 292], where each
chunk holds rows 71k..71k+70 as two field segments [2 guard | 142 | 2 guard].
All matmul rhs operands are contiguous 292-column runs (N=292 >= 256 so
float32r matmuls stream at 1 cycle/row).

Per-core per-step compute:
  PSUM_m = band_x @ p1          (x-stencil + diag + 2I; 2 K-chunks; +32 meas
                                 selection rows augmented onto chunk-0 lhsT)
         + a*I @ p1(cols+-1) + b*I @ p1(cols+-2)    (y-stencil, shifted rhs)
         + I @ G[j]                                  (host-precomputed rf*d2)
  p_new  = PSUM_m - p0          (DVE fused copyback, rotates state tiles)
  meas   = per-field one-hot mask-reduce (STT accum) of the selection rows.
"""
import sys
import os
import numpy as np
from contextlib import ExitStack

sys.path.insert(0, "/opt/trn_rl_repo")

# ---- problem constants (hardcoded; kernel.py must be self-contained) ----
NX = 192
NT = 200
dtime = 0.3
nm, sR = 32, 70
bg = 1.5
LO, HI = 25, 167            # interior rows/cols [LO, HI) -> D = 142
D = HI - LO
CLO, CHI = 48, 144          # central rf-support window (96 wide)
CW = CHI - CLO
COFF = CLO - LO             # 23: central window offset inside domain
C = (dtime * bg / 1.0) ** 2  # 0.2025
K = 71                      # row-chunk size (2 chunks of 71 = 142)
SEG = 2 + D + 2             # 146: per-field segment with 2-col guards
CHW = 2 * SEG               # 292: chunk width (two fields)
NRR = 8
BB = 2
NMEAS = nm

_thetas = 2 * np.pi * np.arange(nm) / nm
_MX = (NX / 2 + sR * np.cos(_thetas)).astype(int)
_MY = (NX / 2 + sR * np.sin(_thetas)).astype(int)

INCLUDE_2I = True           # fold the 2*p1 term into the band matmul

_prog_cache = {}


def _build_band_consts():
    """Host-side constant matrices for the matmuls (numpy float32)."""
    S = np.zeros((D, D), np.float32)
    idx = np.arange(D)
    S[idx, idx] = -60.0 * C / 12.0 + (2.0 if INCLUDE_2I else 0.0)
    S[idx[:-1], idx[:-1] + 1] = 16.0 * C / 12.0
    S[idx[1:], idx[1:] - 1] = 16.0 * C / 12.0
    S[idx[:-2], idx[:-2] + 2] = -C / 12.0
    S[idx[2:], idx[2:] - 2] = -C / 12.0

    BD = {}
    for kc in range(2):
        for mc in range(2):
            blk = S[mc * K:(mc + 1) * K, kc * K:(kc + 1) * K].T.copy()
            if mc == 0:
                aug = np.zeros((K, 96 + NMEAS), np.float32)
                aug[:, :K] = blk
                for i in range(NMEAS):
                    g = _MX[i] - LO
                    if g // K == kc:
                        aug[g % K, 96 + i] = 1.0
                blk = aug
            BD[(kc, mc)] = np.ascontiguousarray(blk)

    IG = np.eye(K, dtype=np.float32)
    SH1 = np.eye(K, dtype=np.float32) * np.float32(16.0 * C / 12.0)
    SH2 = np.eye(K, dtype=np.float32) * np.float32(-C / 12.0)

    # per-field one-hot masks over the 142 data cols
    MASK = np.zeros((NMEAS, D), np.float32)
    for i in range(NMEAS):
        MASK[i, _MY[i] - LO] = 1.0
    return BD, SH1, SH2, IG, MASK


def _build_program(nt=NT, debug=False, reps=1):
    import concourse.bacc as bacc
    import concourse.tile as tile
    import concourse.mybir as mybir

    dt = mybir.dt
    nc = bacc.Bacc("TRN2", target_bir_lowering=False)

    G_d = nc.dram_tensor("G", (NT, BB, CW, CW), dt.float32r, kind="ExternalInput")
    BD_d = {
        (kc, mc): nc.dram_tensor(
            f"BD{kc}{mc}", (K, (96 + NMEAS) if mc == 0 else K), dt.float32r,
            kind="ExternalInput")
        for kc in range(2) for mc in range(2)
    }
    SH1_d = nc.dram_tensor("SH1", (K, K), dt.float32r, kind="ExternalInput")
    SH2_d = nc.dram_tensor("SH2", (K, K), dt.float32r, kind="ExternalInput")
    IG_d = nc.dram_tensor("IG", (K, K), dt.float32r, kind="ExternalInput")
    MASK_d = nc.dram_tensor("MASK", (NMEAS, D), dt.float32, kind="ExternalInput")
    ZERO_d = nc.dram_tensor("ZERO", (K, 300), dt.float32r, kind="ExternalInput")
    OUT_d = nc.dram_tensor("OUT", (BB, NMEAS, NT), dt.float32, kind="ExternalOutput")
    if debug:
        DBGC_d = nc.dram_tensor("DBGC", (2, K, 300), dt.float32, kind="ExternalOutput")
        DBGP_d = nc.dram_tensor("DBGP", (2, K, 300), dt.float32, kind="ExternalOutput")

    GPF = 3  # G stream ring depth
    PAD = 4  # left/right pad so shift offsets stay in-bounds

    with tile.TileContext(nc) as tc, ExitStack() as ctx:
        def sbuf(name, shape, dty):
            return ctx.enter_context(nc.sbuf_tensor(name, shape, dty))

        # per-chunk state tiles: [4 pad | 2 x (2+142+2) | 4 pad] = 300 cols
        PA = [sbuf(f"PA{kc}", [K, 300], dt.float32r) for kc in range(2)]
        PB = [sbuf(f"PB{kc}", [K, 300], dt.float32r) for kc in range(2)]
        # G ring: per chunk, state layout (zero-padded, central cols DMA'd)
        Gr = [[sbuf(f"Gr{i}_{kc}", [K, 300], dt.float32r) for kc in range(2)]
              for i in range(GPF)]
        bd_t = {km: sbuf(f"bd{km[0]}{km[1]}",
                         [K, (96 + NMEAS) if km[1] == 0 else K], dt.float32r)
                for km in BD_d}
        sh1_t = sbuf("sh1", [K, K], dt.float32r)
        sh2_t = sbuf("sh2", [K, K], dt.float32r)
        ig_t = sbuf("ig", [K, K], dt.float32r)
        mask_t = sbuf("mask", [NMEAS, D], dt.float32)
        meas_t = sbuf("meas", [NMEAS, BB * NT], dt.float32)
        scr_t = sbuf("scr", [NMEAS, D], dt.float32)

        ps_pool = ctx.enter_context(tc.tile_pool(name="ps", bufs=2, space="PSUM"))

        for kc in range(2):
            nc.sync.dma_start(PA[kc][:], ZERO_d[:])
            nc.sync.dma_start(PB[kc][:], ZERO_d[:])
        for i in range(GPF):
            for kc in range(2):
                nc.sync.dma_start(Gr[i][kc][:], ZERO_d[:])
        nc.vector.memset(meas_t[:], 0.0)
        for km, d in BD_d.items():
            nc.sync.dma_start(bd_t[km][:], d[:])
        nc.sync.dma_start(sh1_t[:], SH1_d[:])
        nc.sync.dma_start(sh2_t[:], SH2_d[:])
        nc.sync.dma_start(ig_t[:], IG_d[:])
        nc.sync.dma_start(mask_t[:], MASK_d[:])

        def g_dma(j):
            """DMA G[j] (BB, 96, 96) into ring slot j%GPF, chunk-aligned."""
            for kc in range(2):
                gt = Gr[j % GPF][kc]
                plo = COFF if kc == 0 else 0          # partition base
                rlo = 0 if kc == 0 else 48            # central row base
                src = G_d[j, :, rlo:rlo + 48, :].rearrange("f r c -> r f c")
                dst = gt[plo:plo + 48, PAD:PAD + CHW].rearrange(
                    "p (f c) -> p f c", c=SEG)[:, :, 2 + COFF:2 + COFF + CW]
                nc.sync.dma_start(dst, src)

        def run_view(t, off=0):
            """Contiguous [71, 292] matmul-rhs view at col-tap off."""
            return t[:, PAD + off: PAD + off + CHW]

        def data_view(t, cast_f32=False):
            """[71, 2(field), 142] data view (for DVE ops)."""
            v = t[:, PAD:PAD + CHW]
            if cast_f32:
                v = v.bitcast(dt.float32)
            return v.rearrange("p (f c) -> p f c", c=SEG)[:, :, 2:2 + D]

        def central_view(t, cast_f32=False):
            """[71, 2(field), 96] central-cols view of a state chunk tile."""
            v = t[:, PAD:PAD + CHW]
            if cast_f32:
                v = v.bitcast(dt.float32)
            return v.rearrange("p (f c) -> p f c", c=SEG)[
                :, :, 2 + COFF:2 + COFF + CW]

        def meas_extract(pt, j):
            """Extract 32x2 measurements for output step j from selection rows."""
            for f in range(2):
                seg = pt[96:96 + NMEAS, f * SEG + 2: f * SEG + 2 + D]
                nc.vector.scalar_tensor_tensor(
                    out=scr_t[:], in0=seg, scalar=1.0, in1=mask_t[:],
                    op0=mybir.AluOpType.mult, op1=mybir.AluOpType.mult,
                    accum_out=meas_t[:, f * NT + j: f * NT + j + 1],
                )

        cur, prev = PA, PB
        for rep in range(reps):
          if rep > 0:
            # re-zero state so values stay bounded across timing reps
            for kc in range(2):
                nc.sync.dma_start(PA[kc][:], ZERO_d[:])
                nc.sync.dma_start(PB[kc][:], ZERO_d[:])
          for j in range(nt):
              if j == 0:
                  for q in range(min(GPF - 1, nt)):
                      g_dma(q)
              if j + GPF - 1 < nt:
                  g_dma(j + GPF - 1)

              psums = []
              for mc in range(2):
                  mrows = 128 if mc == 0 else K
                  pt = ps_pool.tile([mrows, CHW], dt.float32, tag=f"ps{mc}")
                  pd = pt[0:K, :]
                  full = pt[:] if mc == 0 else pd
                  nc.tensor.matmul(full, bd_t[(0, mc)][:], run_view(cur[0]),
                                   start=True, stop=False)
                  nc.tensor.matmul(pd, sh1_t[:], run_view(cur[mc], -1),
                                   start=False, stop=False)
                  nc.tensor.matmul(pd, sh1_t[:], run_view(cur[mc], 1),
                                   start=False, stop=False)
                  nc.tensor.matmul(pd, sh2_t[:], run_view(cur[mc], -2),
                                   start=False, stop=False)
                  nc.tensor.matmul(pd, sh2_t[:], run_view(cur[mc], 2),
                                   start=False, stop=False)
                  nc.tensor.matmul(pd, ig_t[:], run_view(Gr[j % GPF][mc]),
                                   start=False, stop=False)
                  nc.tensor.matmul(full, bd_t[(1, mc)][:], run_view(cur[1]),
                                   start=False, stop=True)
                  psums.append(pt)

              for mc in range(2):
                  pd = psums[mc][0:K, :].rearrange(
                      "p (f c) -> p f c", c=SEG)[:, :, 2:2 + D]
                  nc.vector.tensor_tensor(
                      out=data_view(prev[mc]), in0=pd,
                      in1=data_view(prev[mc], cast_f32=True),
                      op=mybir.AluOpType.subtract)

              if j > 0:
                  meas_extract(psums[0], j - 1)

              cur, prev = prev, cur

        # final measurement for output step nt-1 on the final state
        pt = ps_pool.tile([128, CHW], dt.float32, tag="ps0")
        nc.tensor.matmul(pt[:], bd_t[(0, 0)][:], run_view(cur[0]),
                         start=True, stop=False)
        nc.tensor.matmul(pt[:], bd_t[(1, 0)][:], run_view(cur[1]),
                         start=False, stop=True)
        meas_extract(pt, nt - 1)

        if debug:
            for kc in range(2):
                nc.sync.dma_start(DBGC_d[kc], cur[kc][:].bitcast(dt.float32))
                nc.sync.dma_start(DBGP_d[kc], prev[kc][:].bitcast(dt.float32))
        nc.sync.dma_start(
            OUT_d[:].rearrange("f i j -> i f j"),
            meas_t[:].rearrange("i (f j) -> i f j", j=NT))

    nc.compile()
    return nc


def kernel(x, P0):
    x = np.asarray(x, dtype=np.float32)
    P0 = np.asarray(P0, dtype=np.float32)
    from concourse.bass_utils import run_bass_kernel_spmd

    if "prog" not in _prog_cache:
        _prog_cache["prog"] = _build_program()
    nc = _prog_cache["prog"]

    BD, SH1, SH2, IG, MASK = _build_band_consts()

    xx = bg / x[:, 0]
    rf = (1.0 - xx * xx).astype(np.float32)           # (B, 96, 96)
    P0c = P0[0, :, :, CLO:CHI, CLO:CHI]               # (NR, NT, 96, 96)
    d2 = np.zeros_like(P0c)
    d2[:, 2:] = P0c[:, 2:] - 2.0 * P0c[:, 1:-1] + P0c[:, :-2]

    consts = {"SH1": SH1, "SH2": SH2, "IG": IG, "MASK": MASK,
              "ZERO": np.zeros((K, 300), np.float32)}
    for km, v in BD.items():
        consts[f"BD{km[0]}{km[1]}"] = v

    in_maps = []
    for r in range(NRR):
        G = (rf[None, :, :, :] * d2[r][:, None, :, :]).astype(np.float32)
        m = dict(consts)
        m["G"] = np.ascontiguousarray(G)
        in_maps.append(m)

    trace = bool(int(os.environ.get("KERNEL_TRACE", "0")))
    res = run_bass_kernel_spmd(nc, in_maps, core_ids=list(range(NRR)),
                               trace=trace)
    _prog_cache["last_result"] = res
    out = np.zeros((BB, NRR, NMEAS, NT), np.float32)
    for r in range(NRR):
        out[:, r] = res.results[r]["OUT"]
    return out

